# revision 33
# baseline (speedup 1.0000x reference)
"""Trainium2 Bass kernel: transformer encoder layer (DeepPM style).

B=8 batch elements sharded 1-per-core across 8 NeuronCores.
Per core everything is computed feature-major ("T layout": [d, token])
so no activation transposes are needed until the very end:

  - QKV proj:   lhsT = W.T (host-transposed), rhs = x.T
  - scores.T[k,q] per head via PE (K=32 contraction)
  - softmax without max-subtraction: exp on ACT, additive mask folded in
    multiplicatively (attn = exp(scale*qk) * E, E = exp(mask) host-built)
  - denominator via ones-column appended to V (row 32 of ctx psum)
  - ctx.T = V_aug.T @ attn  (lhsT = token-major V tile)
  - per-head normalize: reciprocal + selector-matmul broadcast
  - out/proj collapsed into one matmul (Wc = proj_w @ out_w, host-folded
    biases), fp32 residual, FFN with fused gelu+bias on ACT
  - final PE transpose to token-major with padded-row zeroing fused into
    the psum->sbuf copy (scale = 0/1 per-partition mask)
"""

import numpy as np
import ml_dtypes
from contextlib import ExitStack

BF16 = ml_dtypes.bfloat16
F8 = ml_dtypes.float8_e4m3
F32 = np.float32

B, L, D, H, DFF = 8, 1024, 256, 8, 2048
DH = D // H          # 32
P = 128
NKT = L // P         # 8 token tiles
NDT = D // P         # 2 feature tiles
NF1 = DFF // P       # 16
QCW = 512            # q-chunk width (max moving free dim)
NQC = L // QCW       # 2
NCORES = 8

_BUILT = {}


def _build_module(n_iters: int = 1):
    import concourse.tile as tile
    import concourse.mybir as mybir
    from concourse import bacc
    from concourse.masks import make_identity

    dt = mybir.dt
    AF = mybir.ActivationFunctionType
    OP = mybir.AluOpType

    nc = bacc.Bacc("TRN2", target_bir_lowering=False, debug=False)

    def din(name, shape, dtype):
        return nc.dram_tensor(name, shape, dtype, kind="ExternalInput").ap()

    xtb = din("xtb", [P, NDT, L], dt.float8e4)
    xt32 = din("xt32", [P, NDT, L], dt.float32)
    med = din("med", [P, NKT, P], dt.bfloat16)
    qauxp = din("qauxp", [2, 3, 4, L], dt.bfloat16)
    qauxm = din("qauxm", [2, 3, 4, L], dt.bfloat16)
    kaux = din("kaux", [2, 3, 4, L], dt.bfloat16)
    wqk = din("wqk", [P, NDT, 2 * D], dt.float8e4)
    wv = din("wv", [P, NDT, D], dt.float8e4)
    wc = din("wc", [P, NDT, D], dt.float8e4)
    wf1 = din("wf1", [P, NDT, DFF], dt.float8e4)
    wf2 = din("wf2", [P, NF1, D], dt.float8e4)
    bqk = din("bqk", [P, 4], dt.float32)
    mbf2 = din("mbf2", [P, NDT], dt.float32)
    bf1 = din("bf1", [P, NF1], dt.float32)
    y = nc.dram_tensor("y", [L, D], dt.float32, kind="ExternalOutput").ap()

    with tile.TileContext(nc) as tc, ExitStack() as ctx:
        consts = ctx.enter_context(tc.tile_pool(name="consts", bufs=1))
        acts = ctx.enter_context(tc.tile_pool(name="acts", bufs=1))
        outp = ctx.enter_context(tc.tile_pool(name="outp", bufs=3))
        psum = ctx.enter_context(tc.tile_pool(name="ps", bufs=2, space="PSUM"))
        psS = ctx.enter_context(tc.tile_pool(name="psS", bufs=2, space="PSUM"))
        psC = ctx.enter_context(tc.tile_pool(name="psC", bufs=2, space="PSUM"))

        # ---- constants; critical-path loads first, bulk weights on SWDGE ----
        c_wqk = consts.tile([P, NDT, 2 * D], dt.float8e4, tag="wqk")
        nc.sync.dma_start(out=c_wqk, in_=wqk)
        c_bqk = consts.tile([P, 4], dt.float32, tag="bqk")
        nc.sync.dma_start(out=c_bqk, in_=bqk)
        c_wv = consts.tile([P, NDT, D], dt.float8e4, tag="wv")
        c_id32 = consts.tile([P, P], dt.float32, tag="id32")
        make_identity(nc, c_id32)
        c_idb = consts.tile([P, P], dt.bfloat16, tag="idb")
        make_identity(nc, c_idb)
        c_wc = consts.tile([P, NDT, D], dt.float8e4, tag="wc")
        c_mbf2 = consts.tile([P, NDT], dt.float32, tag="mbf2")
        c_wf1 = consts.tile([P, NDT, DFF], dt.float8e4, tag="wf1")
        c_bf1 = consts.tile([P, NF1], dt.float32, tag="bf1")
        c_wf2 = consts.tile([P, NF1, D], dt.float8e4, tag="wf2")

        for it_ in range(n_iters):
            c_xtb = acts.tile([P, NDT, L], dt.float8e4, tag="xtb")
            nc.sync.dma_start(out=c_xtb, in_=xtb)
            if it_ == 0:
                nc.sync.dma_start(out=c_wv, in_=wv)
            q_p = acts.tile([P, 4, L], dt.bfloat16, tag="qp")
            q_m = acts.tile([P, 4, L], dt.bfloat16, tag="qm_")
            k2 = acts.tile([P, 4, L], dt.bfloat16, tag="k2")
            for r_ in range(3):
                nc.sync.dma_start(
                    out=q_p.rearrange("(g r) t q -> g r t q", r=64)[:, 32 + r_, :, :],
                    in_=qauxp[:, r_, :, :],
                )
                nc.sync.dma_start(
                    out=q_m.rearrange("(g r) t q -> g r t q", r=64)[:, 32 + r_, :, :],
                    in_=qauxm[:, r_, :, :],
                )
                nc.sync.dma_start(
                    out=k2.rearrange("(g r) t q -> g r t q", r=64)[:, 32 + r_, :, :],
                    in_=kaux[:, r_, :, :],
                )
            c_med = acts.tile([P, NKT, P], dt.bfloat16, tag="med")
            nc.sync.dma_start(out=c_med, in_=med)
            c_x32 = acts.tile([P, NDT, L], dt.float32, tag="x32")
            nc.sync.dma_start(out=c_x32, in_=xt32)

            # ---- Q,K projections (feature-major, scale folded into Q) ----
            # Head h at partition base (h%2)*64, free index h//2; row base+32
            # holds the aux contraction row for the separable-mask trick:
            # K aux = +/-1, Q aux = -q/s, so a K=33 matmul adds -+q/s to the
            # scores while the per-k +-k/s rides in the exp bias.
            # fp8 weights x16; the 1/16 descale is fused into the bias add.
            DR = mybir.MatmulPerfMode.DoubleRow
            for mt in (0, 2, 1, 3):
                dst = q_p if mt < 2 else k2
                early = mt in (0, 2)
                if early:
                    # paired 2-bank psum: both q-chunks, one DR matmul
                    ps2 = psS.tile([P, 2 * QCW], dt.float32, tag="score",
                                   name="qkvps")
                    for qc in range(NQC):
                        nc.tensor.matmul(
                            ps2[:, qc * QCW:(qc + 1) * QCW],
                            lhsT=c_wqk[:, :, mt * P:(mt + 1) * P],
                            rhs=c_xtb[:, :, qc * QCW:(qc + 1) * QCW],
                            start=True, stop=True, perf_mode=DR,
                        )
                    for i in range(4):
                        h = (mt % 2) * 4 + i
                        nc.vector.tensor_scalar(
                            out=dst[(h % 2) * 64:(h % 2) * 64 + DH, h // 2, :],
                            in0=ps2[i * DH:(i + 1) * DH, :],
                            scalar1=1.0 / 16.0,
                            scalar2=c_bqk[i * DH:(i + 1) * DH, mt:mt + 1],
                            op0=OP.mult, op1=OP.add,
                        )
                else:
                    for qc in range(NQC):
                        qs = slice(qc * QCW, (qc + 1) * QCW)
                        ps = psum.tile([P, QCW], dt.float32, tag="mm")
                        nc.tensor.matmul(
                            ps,
                            lhsT=c_wqk[:, :, mt * P:(mt + 1) * P],
                            rhs=c_xtb[:, :, qs],
                            start=True, stop=True, perf_mode=DR,
                        )
                        for i in range(4):
                            h = (mt % 2) * 4 + i
                            d_ = dst[(h % 2) * 64:(h % 2) * 64 + DH, h // 2, qs]
                            s_ = ps[i * DH:(i + 1) * DH, :]
                            b_ = c_bqk[i * DH:(i + 1) * DH, mt:mt + 1]
                            nc.vector.tensor_scalar(
                                out=d_, in0=s_, scalar1=1.0 / 16.0,
                                scalar2=b_, op0=OP.mult, op1=OP.add)

            for hf in range(4):
                for g in range(2):
                    nc.gpsimd.tensor_copy(
                        out=q_m[g * 64:g * 64 + DH, hf, :],
                        in_=q_p[g * 64:g * 64 + DH, hf, :],
                    )

            # ---- V (token-major, fp8 x16) + den column (=16, cancels) ----
            # ctx is computed flipped (token-major q partitions): per
            # (q-tile, head) DoubleRow matmuls consume [k, 2kt, 33] slices.
            # issued lazily inside the attention loop (after the first head's
            # scores) so the first exps aren't delayed.
            vzz = acts.tile([P, NKT, H, DH + 1], dt.float8e4, tag="vzz")
            nc.vector.memset(vzz[:, :, :, DH:DH + 1], 16.0)

            def issue_vaug():
                for tt in range(NKT):
                    ps = psum.tile([P, D], dt.float32, tag="mm")
                    nc.tensor.matmul(
                        ps,
                        lhsT=c_xtb[:, :, tt * P:(tt + 1) * P],
                        rhs=c_wv[:, :, :],
                        start=True, stop=True, perf_mode=DR,
                    )
                    nc.vector.tensor_copy(
                        out=vzz[:, tt, :, 0:DH],
                        in_=ps.rearrange("p (g d) -> p g d", g=H))

            if it_ == 0:
                nc.sync.dma_start(out=c_wc, in_=wc)
                nc.sync.dma_start(out=c_mbf2, in_=mbf2)
                nc.sync.dma_start(out=c_wf1, in_=wf1)
                nc.sync.dma_start(out=c_bf1, in_=bf1)
                nc.sync.dma_start(out=c_wf2, in_=wf2)

            # ---- attention, q-chunk major; per-chunk full tail ----
            from concourse.tile import add_dep_helper
            exp_by = {}
            h32s, hbs = [], []
            NQT = QCW // P  # q-tiles (128 wide) per q-chunk
            vaug_issued = [False]
            for qc in range(NQC):
                qs = slice(qc * QCW, (qc + 1) * QCW)
                # all heads' attention weights for this q-chunk, fp8, laid
                # out so [., 2kt, h, 128q] slices serve as DoubleRow lhsT
                at = acts.tile([P, NKT, H, QCW], dt.float8e4, tag=f"at{qc}",
                               name=f"at{qc}")
                for h in range(H):
                    hb_ = (h % 2) * 64
                    hf = h // 2
                    pr_order = [p_ for p_ in range(4) if p_ // 2 <= qc] + \
                               [p_ for p_ in range(4) if p_ // 2 > qc]
                    for pr in pr_order:
                        ps = psS.tile([P, 2 * QCW], dt.float32, tag="score")
                        in_chunk = (pr // 2 == qc)
                        for j in range(2):
                            kt = 2 * pr + j
                            half = ps[:, j * QCW:(j + 1) * QCW]
                            if not in_chunk:
                                qv = q_p if kt < 4 * qc else q_m
                                nc.tensor.matmul(
                                    half,
                                    lhsT=k2[hb_:hb_ + DH + 3, hf,
                                            kt * P:(kt + 1) * P],
                                    rhs=qv[hb_:hb_ + DH + 3, hf, qs],
                                    start=True, stop=True,
                                )
                                continue
                            # in-chunk k-tile: split columns around the
                            # 128-wide diagonal block; the diag block gets
                            # its full mask bias added via a matmul against
                            # the identity (out += med.T @ I), so no
                            # elementwise mask multiply is needed anywhere
                            jl = kt - 4 * qc
                            q0 = qc * QCW
                            if jl > 0:  # left of diag: q < k -> future
                                nc.tensor.matmul(
                                    half[:, 0:jl * P],
                                    lhsT=k2[hb_:hb_ + DH + 3, hf,
                                            kt * P:(kt + 1) * P],
                                    rhs=q_m[hb_:hb_ + DH + 3, hf,
                                            q0:q0 + jl * P],
                                    start=True, stop=True,
                                )
                            nc.tensor.matmul(
                                half[:, jl * P:(jl + 1) * P],
                                lhsT=k2[hb_:hb_ + DH, hf, kt * P:(kt + 1) * P],
                                rhs=q_p[hb_:hb_ + DH, hf,
                                        q0 + jl * P:q0 + (jl + 1) * P],
                                start=True, stop=False,
                            )
                            nc.tensor.matmul(
                                half[:, jl * P:(jl + 1) * P],
                                lhsT=c_med[:, kt, :],
                                rhs=c_idb,
                                start=False, stop=True,
                            )
                            if jl < 3:  # right of diag: q > k -> past
                                nc.tensor.matmul(
                                    half[:, (jl + 1) * P:],
                                    lhsT=k2[hb_:hb_ + DH + 3, hf,
                                            kt * P:(kt + 1) * P],
                                    rhs=q_p[hb_:hb_ + DH + 3, hf,
                                            q0 + (jl + 1) * P:q0 + QCW],
                                    start=True, stop=True,
                                )
                        dst2 = at[:, 2 * pr:2 * pr + 2, h, :]
                        ei = nc.scalar.activation(
                            out=dst2,
                            in_=ps.rearrange("p (a b) -> p a b", a=2),
                            func=AF.Exp).ins
                        exp_by.setdefault((qc, h), []).append(ei)
                        if not vaug_issued[0]:
                            vaug_issued[0] = True
                            issue_vaug()

                # -- flipped ctx: per (q-tile, head) DoubleRow matmuls give
                # token-major [q, 33] with the denominator in column 32;
                # normalization fuses into the psum->sbuf copy --
                ctxn = acts.tile([P, NQT, D], dt.bfloat16, tag=f"ctxn{qc}",
                                 name=f"ctxn{qc}")
                for j in range(NQT):
                    jw = slice(j * P, (j + 1) * P)
                    pctx = psC.tile([P, H, DH + 1], dt.float32, tag="ctx")
                    for h in range(H):
                        for tp in range(NKT // 2):
                            nc.tensor.matmul(
                                pctx[:, h, :],
                                lhsT=at[:, 2 * tp:2 * tp + 2, h, jw],
                                rhs=vzz[:, 2 * tp:2 * tp + 2, h, :],
                                start=(tp == 0),
                                stop=(tp == NKT // 2 - 1),
                                perf_mode=DR,
                            )
                    dven = acts.tile([P, H], dt.float32, tag=f"dv{qc}{j}",
                                     name="dven")
                    nc.vector.tensor_copy(
                        out=dven, in_=pctx[:, :, DH:DH + 1])
                    recq = acts.tile([P, H], dt.float32, tag=f"rq{qc}{j}",
                                     name="recq")
                    nc.vector.reciprocal(out=recq, in_=dven)
                    for h in range(H):
                        # x64 keeps fp8 ctxn out of the subnormal range
                        nc.vector.tensor_scalar(
                            out=ctxn[:, j, h * DH:(h + 1) * DH],
                            in0=pctx[:, h, 0:DH],
                            scalar1=recq[:, h:h + 1], scalar2=64.0,
                            op0=OP.mult, op1=OP.mult,
                        )

                # -- transpose ctx to feature-major for the out-projection --
                ctxf = acts.tile([P, NDT, QCW], dt.float8e4, tag=f"ctxf{qc}",
                                 name=f"ctxf{qc}")
                for mt in range(NDT):
                    ptf = psC.tile([P, QCW], dt.bfloat16, tag="ctx")
                    for j in range(NQT):
                        nc.tensor.transpose(
                            ptf[:, j * P:(j + 1) * P],
                            ctxn[:, j, mt * P:(mt + 1) * P], c_idb,
                        )
                    nc.vector.tensor_copy(out=ctxf[:, mt, :], in_=ptf)

                # -- combined out+proj matmul (fp8 x128 weights, ctx x64),
                #    fp32 residual; x32 carries src + bc + bf2 host-folded --
                h32 = acts.tile([P, NDT, QCW], dt.float32, tag=f"h32{qc}",
                                name=f"h32{qc}")
                hb = acts.tile([P, NDT, QCW], dt.float8e4, tag=f"hb{qc}",
                               name=f"hb{qc}")
                for mt in range(NDT):
                    ps = psum.tile([P, QCW], dt.float32, tag="mm")
                    nc.tensor.matmul(
                        ps,
                        lhsT=c_wc[:, :, mt * P:(mt + 1) * P],
                        rhs=ctxf[:, :, :],
                        start=True, stop=True, perf_mode=DR,
                    )
                    nc.vector.scalar_tensor_tensor(
                        out=h32[:, mt, :], in0=ps, scalar=1.0 / 8192.0,
                        in1=c_x32[:, mt, qs], op0=OP.mult, op1=OP.add,
                    )
                    nc.gpsimd.tensor_scalar_add(
                        out=hb[:, mt, :], in0=h32[:, mt, :],
                        scalar1=c_mbf2[:, mt:mt + 1])
                if True:
                    h32s.append(h32); hbs.append(hb)

            last_exp = exp_by[(1, H - 1)][-1]
            # ---- FFN + store, per chunk; gelus after all exps ----
            for qc in range(NQC):
                qs = slice(qc * QCW, (qc + 1) * QCW)
                h32, hb = h32s[qc], hbs[qc]
                g = acts.tile([P, NF1, QCW], dt.float8e4, tag=f"g{qc}",
                              name=f"g{qc}")
                for mt in range(NF1):
                    ps = psS.tile([P, QCW], dt.float32, tag="score")
                    nc.tensor.matmul(
                        ps,
                        lhsT=c_wf1[:, :, mt * P:(mt + 1) * P],
                        rhs=hb[:, :, :],
                        start=True, stop=True, perf_mode=DR,
                    )
                    gi = nc.scalar.activation(
                        out=g[:, mt, :], in_=ps,
                        func=AF.Gelu, bias=c_bf1[:, mt:mt + 1],
                        scale=1.0 / 16.0,
                    )
                    add_dep_helper(gi.ins, last_exp, sync=False,
                                   reason="act table: gelu after all exp")
                o32 = acts.tile([P, NDT, QCW], dt.float32, tag=f"o32{qc}",
                                name=f"o32{qc}")
                for mt in range(NDT):
                    ps = psum.tile([P, QCW], dt.float32, tag="mm")
                    for kp in range(NF1 // 2):
                        nc.tensor.matmul(
                            ps,
                            lhsT=c_wf2[:, 2 * kp:2 * kp + 2,
                                       mt * P:(mt + 1) * P],
                            rhs=g[:, 2 * kp:2 * kp + 2, :],
                            start=(kp == 0),
                            stop=(kp == NF1 // 2 - 1),
                            perf_mode=DR,
                        )
                    nc.vector.scalar_tensor_tensor(
                        out=o32[:, mt, :], in0=ps, scalar=1.0 / 16.0,
                        in1=h32[:, mt, :], op0=OP.mult, op1=OP.add,
                    )

                # -- transpose to token-major, store (padded rows zeroed on
                # the host after gather) --
                for tt in range(qc * NKT // NQC, (qc + 1) * NKT // NQC):
                    to = tt - qc * NKT // NQC
                    ot = outp.tile([P, D], dt.float32, tag="ot")
                    for dtt in range(NDT):
                        pt = psC.tile([P, P], dt.float32, tag="ctx")
                        nc.tensor.transpose(
                            pt, o32[:, dtt, to * P:(to + 1) * P], c_id32
                        )
                        nc.vector.tensor_copy(
                            out=ot[:, dtt * P:(dtt + 1) * P], in_=pt,
                        )
                    nc.sync.dma_start(out=y[tt * P:(tt + 1) * P, :], in_=ot)

    nc.compile()
    return nc


def _get_module(n_iters: int = 1):
    if n_iters not in _BUILT:
        _BUILT[n_iters] = _build_module(n_iters)
    return _BUILT[n_iters]


def _rearr(a, nt):
    """[nt*128, F] row-major -> device layout [128, nt, F]."""
    f = a.shape[1]
    return np.ascontiguousarray(a.reshape(nt, P, f).transpose(1, 0, 2))


def prepare_in_maps(inputs):
    src = np.asarray(inputs["src"], F32)
    mask = np.asarray(inputs["src_key_padding_mask"])
    in_proj_w = np.asarray(inputs["in_proj_w"], F32)
    in_proj_b = np.asarray(inputs["in_proj_b"], F32)
    out_w = np.asarray(inputs["out_w"], F32)
    out_b = np.asarray(inputs["out_b"], F32)
    proj_w = np.asarray(inputs["proj_w"], F32)
    proj_b = np.asarray(inputs["proj_b"], F32)
    ff1_w = np.asarray(inputs["ff1_w"], F32)
    ff1_b = np.asarray(inputs["ff1_b"], F32)
    ff2_w = np.asarray(inputs["ff2_w"], F32)
    ff2_b = np.asarray(inputs["ff2_b"], F32)

    scale = 1.0 / np.sqrt(F32(DH))
    wq = in_proj_w[:D] * scale
    bq = in_proj_b[:D] * scale
    wk = in_proj_w[D:2 * D]
    bk = in_proj_b[D:2 * D]
    wv_ = in_proj_w[2 * D:]
    bv = in_proj_b[2 * D:]

    # fp8 weights, scaled up out of the e4m3 subnormal range; descales are
    # fused into the downstream bias/residual ops on device
    wqk_dev = _rearr(np.concatenate([wq, wk], 0).T * 16, NDT).astype(F8)
    wv_dev = _rearr(wv_.T * 16, NDT).astype(F8)
    wc_mat = proj_w @ out_w
    wc_dev = _rearr(wc_mat.T * 128, NDT).astype(F8)
    bo2 = out_b + out_w @ bv
    bc_vec = proj_w @ bo2 + proj_b
    wf1_dev = _rearr(ff1_w.T * 16, NDT).astype(F8)
    wf2_dev = _rearr(ff2_w.T * 16, NF1).astype(F8)

    bqk_dev = np.ascontiguousarray(
        np.concatenate([bq, bk]).reshape(4, P).T).astype(F32)
    bf1_dev = np.ascontiguousarray(ff1_b.reshape(NF1, P).T).astype(F32)
    mbf2_dev = np.ascontiguousarray(-ff2_b.reshape(NDT, P).T).astype(F32)

    shared = {
        "wqk": wqk_dev, "wv": wv_dev, "wc": wc_dev,
        "wf1": wf1_dev, "wf2": wf2_dev,
        "bqk": bqk_dev, "mbf2": mbf2_dev, "bf1": bf1_dev,
    }
    # residual carries src + bc + bf2 (bf2 subtracted back out before FFN1)
    xadd = (bc_vec + ff2_b).astype(F32)

    ki = np.arange(L, dtype=F32)[:, None]
    qi = np.arange(L, dtype=F32)[None, :]
    dist = np.abs(qi - ki)

    in_maps = []
    for b in range(NCORES):
        s = int((~mask[b]).sum())
        xT = src[b].T  # [D, L]
        # med[a, kt, b]: mask bias for the 128-wide diagonal block of k-tile
        # kt, added into the scores psum via matmul against the identity
        # (out[k^, q^] += med[q^, kt, k^]); 1 - |q-k|/s, -1e5 on invalid pairs
        ii = np.arange(P, dtype=np.float64)
        dloc = np.abs(ii[:, None] - ii[None, :])      # [a, b]
        med_l = []
        for kt in range(NKT):
            gl = kt * P + ii
            inv = (gl >= s)
            mm_ = 1.0 - dloc / s - 1e5 * (inv[:, None] | inv[None, :])
            med_l.append(mm_)
        med_dev = np.ascontiguousarray(
            np.stack(med_l, axis=1)).astype(BF16)     # [P(a), NKT, P(b)]
        kvec = np.arange(L, dtype=np.float64)
        # aux rows (r32, r33, r34) broadcast over (group, head-slot):
        #   k side:  [1, k/s, 1 + pad(k)*(-1e5)]
        #   q side +: [-q/s, +1, +1]   q side -: [+q/s, -1, +1]
        pad_k = (kvec >= s) * (-1e5)
        kaux3 = np.stack([np.ones(L), kvec / s, 1.0 + pad_k], axis=0)
        qp3 = np.stack([-kvec / s, np.ones(L), np.ones(L)], axis=0)
        qm3 = np.stack([kvec / s, -np.ones(L), np.ones(L)], axis=0)

        def _aux(a):
            return np.ascontiguousarray(
                np.broadcast_to(a[None, :, None, :], (2, 3, 4, L))).astype(BF16)

        im = dict(shared)
        im["xtb"] = _rearr(xT, NDT).astype(F8)
        im["xt32"] = _rearr(xT + xadd[:, None], NDT).astype(F32)
        im["med"] = med_dev
        im["qauxp"] = _aux(qp3)
        im["qauxm"] = _aux(qm3)
        im["kaux"] = _aux(kaux3)
        in_maps.append(im)
    return in_maps


def run_on_device(inputs, n_iters: int = 1, trace: bool = False):
    from concourse import bass_utils
    nc = _get_module(n_iters)
    in_maps = prepare_in_maps(inputs)
    res = bass_utils.run_bass_kernel_spmd(
        nc, in_maps, core_ids=list(range(NCORES)), trace=trace)
    return res


def kernel(**inputs) -> np.ndarray:
    res = run_on_device(inputs)
    out = np.stack([res.results[b]["y"] for b in range(NCORES)], axis=0)
    out = out.astype(F32)
    # zero padded query rows (masked_fill with all -inf rows in the ref)
    mask = np.asarray(inputs["src_key_padding_mask"])
    for b in range(NCORES):
        s = int((~mask[b]).sum())
        out[b, s:, :] = 0.0
    return out

